# revision 1
# baseline (speedup 1.0000x reference)
"""Trainium2 Bass kernel for nn_GTLayer (sparse_attention problem).

Key structural fact about the reference: H == 1 and the softmax is taken
over the HEAD axis, so softmax(attn, axis=0) on a (1, N, N) tensor is
identically 1.0.  Therefore attn @ v reduces to broadcasting the column
sums of v to every row: the A mask, q and k projections are all dead
code.  The attention output row is a single constant vector

    base = (sum_i h_i) @ vw + N * vb, then @ ow + ob

which we compute exactly on the host.  Folding both BatchNorms (eval
mode -> per-feature affine) and the residuals, the whole layer is

    y = h2 + relu(h2 @ W1 + b1) @ W2 + C        (per-feature constants)

with h2 = h * sP.  The large constant part of t = relu(h2 @ W1 + b1) is
tc = relu(b1) (h2 is zero-mean): the device computes tv = t - tc in bf16
(small values -> accurate) and the exact tc @ W2 + C contribution rides
in the f32 h2C tensor, added on the vector engine.

Device pipeline per core (1024 rows):
  mm1:  zT = W1^T @ h2T            (PE, bf16, psum f32)
  ACT:  u  = relu(z + b1)          (per-partition bias, psum -> sbuf f32)
  DVE:  tv = u - tc  -> bf16
  mm2:  F  = tv @ W2               (PE, bf16, psum f32)
  DVE:  y  = F + h2C               (psum + sbuf f32)
  DMA out.

Rows (N=8192) are sharded over the 8 cores; weights are replicated.
DMA emission order puts row-group-0 activations and W1/W2 first so the
PE can start ~6us in; a chain of tiny warm-up matmuls keeps the PE HAM
unthrottled during the load phase.
"""

import numpy as np
from contextlib import ExitStack

import ml_dtypes
import concourse.bass as bass
import concourse.mybir as mybir
import concourse.tile as tile
from concourse import bacc
from concourse.bass_utils import run_bass_kernel_spmd

N = 8192
D = 512
H1 = 1024
NCORES = 8
RPC = N // NCORES  # rows per core
EPS = 1e-5
N_WARMUP = 7

BF16 = mybir.dt.bfloat16
F32 = mybir.dt.float32
NPBF16 = np.dtype(ml_dtypes.bfloat16)


def build_bass():
    nc = bacc.Bacc(
        "TRN2", target_bir_lowering=False, debug=False, num_devices=NCORES
    )
    h2T = nc.dram_tensor("h2t", [D, RPC], BF16, kind="ExternalInput")
    h2C = nc.dram_tensor("h2c", [RPC, D], F32, kind="ExternalInput")
    W1 = nc.dram_tensor("w1", [D, H1], BF16, kind="ExternalInput")
    W2 = nc.dram_tensor("w2", [H1, D], BF16, kind="ExternalInput")
    # b1 (cols 0..7) and tc (cols 8..15) packed: one DMA trigger
    BC = nc.dram_tensor("bc", [128, 2 * (H1 // 128)], F32, kind="ExternalInput")
    Y = nc.dram_tensor("y", [RPC, D], F32, kind="ExternalOutput")

    NC1 = H1 // 128  # 8 n-chunks in mm1 / k-chunks in mm2
    KC1 = D // 128   # 4 k-chunks in mm1
    RT = RPC // 128  # 8 row tiles
    RG = RPC // 512  # 2 row groups (mm1 free dim 512)

    with ExitStack() as ctx:
        tc = ctx.enter_context(tile.TileContext(nc))
        consts = ctx.enter_context(tc.tile_pool(name="consts", bufs=1))
        acts = ctx.enter_context(tc.tile_pool(name="acts", bufs=1))
        zpsum = ctx.enter_context(tc.tile_pool(name="zpsum", bufs=2, space="PSUM"))
        fpsum = ctx.enter_context(tc.tile_pool(name="fpsum", bufs=4, space="PSUM"))
        wpsum = ctx.enter_context(tc.tile_pool(name="wpsum", bufs=1, space="PSUM"))
        upool = ctx.enter_context(tc.tile_pool(name="upool", bufs=3))
        ypool = ctx.enter_context(tc.tile_pool(name="ypool", bufs=3))

        # --- PE warm-up on a memset tile: no DMA dependency, so the PE's
        # HAM activity window fills right after the preamble and real
        # matmuls run at 2.4 GHz instead of 1.2.
        wa = consts.tile([128, 512], BF16)
        nc.vector.memset(wa[:], 0.0)
        wp = wpsum.tile([128, 512], F32)
        for _ in range(N_WARMUP):
            nc.tensor.matmul(wp[:], wa[:, :128], wa[:], start=True, stop=True)

        # --- streaming inputs, critical-path order, few triggers ----------
        # each dma_start costs ~650ns serial trigger time on its engine's
        # queue; spread non-critical ones across otherwise-idle queues.
        bcsb = consts.tile([128, 2 * NC1], F32)
        nc.sync.dma_start(bcsb[:], BC[:, :])
        b1sb = bcsb[:, 0:NC1]
        tcsb = bcsb[:, NC1 : 2 * NC1]

        H2Tr = h2T.rearrange("(kc p) r -> p kc r", p=128)
        h2tsb = acts.tile([128, KC1, RPC], BF16)
        for kc in range(KC1):  # row-group 0 first: halves the critical load
            nc.sync.dma_start(h2tsb[:, kc, 0:512], H2Tr[:, kc, 0:512])
        w1sb = consts.tile([128, KC1, H1], BF16)
        W1r = W1.rearrange("(kc p) n -> p kc n", p=128)
        for nci in range(NC1):
            nc.sync.dma_start(
                w1sb[:, :, nci * 128 : (nci + 1) * 128],
                W1r[:, :, nci * 128 : (nci + 1) * 128],
            )
        for kc in range(KC1):  # row-group 1 activations
            nc.sync.dma_start(h2tsb[:, kc, 512:RPC], H2Tr[:, kc, 512:RPC])
        # W2 / h2C are needed later: keeping their triggers BEHIND the
        # critical h2T/W1 triggers on the same sync queue throttles them
        # (~650ns serial trigger each), so the critical transfers get the
        # HBM bandwidth first.  (Issuing them in parallel from the idle
        # gpsimd/scalar queues was measurably worse.)
        w2sb = consts.tile([128, NC1, D], BF16)
        W2r = W2.rearrange("(kc p) n -> p kc n", p=128)
        for nci in range(NC1):
            nc.sync.dma_start(w2sb[:, nci, :], W2r[:, nci, :])
        h2csb = acts.tile([128, RT, D], F32)
        H2Cr = h2C.rearrange("(rt p) f -> p rt f", p=128)
        for rt in range(RT):
            nc.sync.dma_start(h2csb[:, rt, :], H2Cr[:, rt, :])
        Yr = Y.rearrange("(rt p) f -> rt p f", p=128)

        # tv stored transposed: [n-in-chunk, n-chunk, row], bf16
        tvsb = acts.tile([128, NC1, RPC], BF16)

        for rg in range(RG):
            rs = rg * 512
            for nci in range(NC1):
                zp = zpsum.tile([128, 512], F32, tag="zp")
                for kc in range(KC1):
                    nc.tensor.matmul(
                        zp[:],
                        w1sb[:, kc, nci * 128 : (nci + 1) * 128],
                        h2tsb[:, kc, rs : rs + 512],
                        start=(kc == 0),
                        stop=(kc == KC1 - 1),
                    )
                u = upool.tile([128, 512], F32, tag="u")
                nc.scalar.activation(
                    u[:],
                    zp[:],
                    mybir.ActivationFunctionType.Relu,
                    bias=b1sb[:, nci : nci + 1],
                    scale=1.0,
                )
                nc.vector.tensor_scalar(
                    tvsb[:, nci, rs : rs + 512],
                    u[:],
                    tcsb[:, nci : nci + 1],
                    None,
                    mybir.AluOpType.subtract,
                )
            for rt in range(rg * (RT // RG), (rg + 1) * (RT // RG)):
                fp = fpsum.tile([128, D], F32, tag="fp")
                for nci in range(NC1):
                    nc.tensor.matmul(
                        fp[:],
                        tvsb[:, nci, rt * 128 : (rt + 1) * 128],
                        w2sb[:, nci, :],
                        start=(nci == 0),
                        stop=(nci == NC1 - 1),
                    )
                ysb = ypool.tile([128, D], F32, tag="ysb")
                nc.vector.tensor_tensor(
                    ysb[:], fp[:], h2csb[:, rt, :], mybir.AluOpType.add
                )
                nc.sync.dma_start(Yr[rt], ysb[:])
    nc.compile()
    return nc


_CACHE = {}


def _get_bass():
    if "nc" not in _CACHE:
        _CACHE["nc"] = build_bass()
    return _CACHE["nc"]


def _host_fold(inputs):
    """Fold attention shortcut + BNs into W1, b1, W2, h2, h2C (float64)."""
    f = lambda k: inputs[k].astype(np.float64)
    h = f("h")
    a1 = f("bn1_g") / np.sqrt(f("bn1_v") + EPS)
    c1 = f("bn1_b") - f("bn1_m") * a1
    a2 = f("bn2_g") / np.sqrt(f("bn2_v") + EPS)
    c2 = f("bn2_b") - f("bn2_m") * a2

    hs = h.sum(axis=0)
    s = hs @ f("vw") + N * f("vb")          # column sums of v
    base = s @ f("ow") + f("ob")            # constant attention-out row
    d1 = base * a1 + c1                     # constant row of bn1(x)
    sP = a1 * a2

    W1 = (1.0 / a2)[:, None] * f("f1w")
    b1 = (d1 @ f("f1w") + f("f1b")).astype(np.float32)
    W2 = f("f2w") * a2[None, :]
    C = (d1 + f("f2b")) * a2 + c2

    # device computes tv = relu(z + b1_f32) - tc_f32 in f32, so use the
    # exact same f32 constants when folding tc @ W2 into h2C
    tc = np.maximum(b1, 0.0)
    Cfull = C + tc.astype(np.float64) @ W2

    h2 = h * sP[None, :]
    pack = lambda v: v.reshape(H1 // 128, 128).T
    return {
        "W1": W1.astype(NPBF16),
        "bc": np.ascontiguousarray(np.concatenate([pack(b1), pack(tc)], axis=1)),
        "W2": W2.astype(NPBF16),
        "h2": h2.astype(np.float32),
        "h2C": (h2 + Cfull[None, :]).astype(np.float32),
    }


def make_in_maps(inputs):
    hf = _host_fold(inputs)
    h2bf = hf["h2"].astype(NPBF16)
    in_maps = []
    for c in range(NCORES):
        r0 = c * RPC
        in_maps.append(
            {
                "h2t": np.ascontiguousarray(h2bf[r0 : r0 + RPC].T),
                "h2c": hf["h2C"][r0 : r0 + RPC],
                "w1": hf["W1"],
                "w2": hf["W2"],
                "bc": hf["bc"],
            }
        )
    return in_maps


def kernel(**inputs):
    nc = _get_bass()
    in_maps = make_in_maps(inputs)
    res = run_bass_kernel_spmd(nc, in_maps, core_ids=list(range(NCORES)))
    return np.concatenate([r["y"] for r in res.results], axis=0)



# revision 2
# speedup vs baseline: 1.5404x; 1.5404x over previous
"""Trainium2 Bass kernel for nn_GTLayer (sparse_attention problem).

Key structural fact about the reference: H == 1 and the softmax is taken
over the HEAD axis, so softmax(attn, axis=0) on a (1, N, N) tensor is
identically 1.0.  Therefore attn @ v reduces to broadcasting the column
sums of v to every row: the A mask, q and k projections are all dead
code.  The attention output row is a single constant vector

    base = (sum_i h_i) @ vw + N * vb, then @ ow + ob

which we compute exactly on the host.  Folding both BatchNorms (eval
mode -> per-feature affine) and the residuals, the whole layer is

    y = h2 + relu(h2 @ W1 + b1) @ W2 + C        (per-feature constants)

with h2 = h * sP.  The large constant part of t = relu(h2 @ W1 + b1) is
tc = relu(b1) (h2 is zero-mean): the device computes tv = t - tc in fp8
(small values -> accurate) and the exact tc @ W2 + C contribution rides
in the bf16 h2ct tensor, added on the vector engine.  The final output
norm is dominated by the constant row (|y|_rms ~ 144), so fp8 matmul
noise lands at ~2e-3 relative - 10x under the 2e-2 gate.

Device pipeline per core (1024 rows, all matmuls fp8 e4m3 DoubleRow,
2x PE throughput):
  mm1:  zT = W1^T @ h2T            (PE, fp8 DR, psum f32) per j-chunk
  ACT:  u  = relu(z + b1)          (per-partition bias, psum -> sbuf f32)
  DVE:  tv = u - tc  -> fp8
  mm2:  fT = W2^T @ tv             (PE, fp8 DR; W2 stationary, output
                                    TRANSPOSED [d, row])
  DVE:  yT = fT + h2ct             (psum + sbuf bf16 -> bf16)
  DMA out (yT, bf16; host transposes back and upcasts to f32).

Rows (N=8192) are sharded over the 8 cores; weights are replicated.
Input DMAs are coalesced into 6 triggers issued critical-first on the
sync queue; a chain of warm-up matmuls keeps the PE HAM busy from t~=0
so the real matmuls run at 2.4 GHz.
"""

import numpy as np
from contextlib import ExitStack

import ml_dtypes
import concourse.bass as bass
import concourse.mybir as mybir
import concourse.tile as tile
from concourse import bacc
from concourse.bass_utils import run_bass_kernel_spmd

N = 8192
D = 512
H1 = 1024
NCORES = 8
RPC = N // NCORES  # rows per core
EPS = 1e-5
N_WARMUP = 8

BF16 = mybir.dt.bfloat16
F32 = mybir.dt.float32
FP8 = mybir.dt.float8e4
NPBF16 = np.dtype(ml_dtypes.bfloat16)
NPFP8 = np.dtype(ml_dtypes.float8_e4m3)
DR = mybir.MatmulPerfMode.DoubleRow

KC = D // 128    # 4 k-chunks in mm1 (2 DoubleRow pairs)
NC1 = H1 // 128  # 8 j-chunks of H1 (4 DoubleRow pairs in mm2)
DS = D // 128    # 4 d-slices of the transposed mm2 output
RG = RPC // 512  # 2 row groups (matmul moving free dim 512)


def build_bass():
    nc = bacc.Bacc(
        "TRN2", target_bir_lowering=False, debug=False, num_devices=NCORES
    )
    h2T = nc.dram_tensor("h2t", [D, RPC], FP8, kind="ExternalInput")
    W1 = nc.dram_tensor("w1", [D, H1], FP8, kind="ExternalInput")
    W2 = nc.dram_tensor("w2", [H1, D], FP8, kind="ExternalInput")
    H2CT = nc.dram_tensor("h2ct", [D, RPC], BF16, kind="ExternalInput")
    # b1 (cols 0..7) and tc (cols 8..15) packed: one DMA trigger
    BC = nc.dram_tensor("bc", [128, 2 * NC1], F32, kind="ExternalInput")
    YT = nc.dram_tensor("yt", [D, RPC], BF16, kind="ExternalOutput")

    with ExitStack() as ctx:
        tc = ctx.enter_context(tile.TileContext(nc))
        consts = ctx.enter_context(tc.tile_pool(name="consts", bufs=1))
        acts = ctx.enter_context(tc.tile_pool(name="acts", bufs=1))
        zpsum = ctx.enter_context(tc.tile_pool(name="zpsum", bufs=2, space="PSUM"))
        fpsum = ctx.enter_context(tc.tile_pool(name="fpsum", bufs=2, space="PSUM"))
        upool = ctx.enter_context(tc.tile_pool(name="upool", bufs=2))
        ypool = ctx.enter_context(tc.tile_pool(name="ypool", bufs=2))

        # --- PE warm-up on a memset tile: no DMA dependency, so the PE's
        # HAM activity window fills right after the preamble and real
        # matmuls run at 2.4 GHz instead of 1.2.  Shares a PSUM bank with
        # the (late-used) mm2 accumulator.
        wa = consts.tile([128, 512], BF16)
        nc.vector.memset(wa[:], 0.0)
        wp = fpsum.tile([128, 512], F32, tag="f0")
        for _ in range(N_WARMUP):
            nc.tensor.matmul(wp[:], wa[:, :128], wa[:], start=True, stop=True)

        # --- streaming inputs, critical-path order, few triggers ----------
        # single sync queue so the critical transfers get HBM bandwidth
        # first; each trigger costs ~650ns serial on the queue.
        w1sb = consts.tile([128, KC, H1], FP8)
        W1r = W1.rearrange("(kc p) j -> p kc j", p=128)
        nc.sync.dma_start(w1sb[:, :, 0:512], W1r[:, :, 0:512])
        h2tsb = acts.tile([128, KC, RPC], FP8)
        H2Tr = h2T.rearrange("(kc p) r -> p kc r", p=128)
        nc.sync.dma_start(h2tsb[:], H2Tr[:])
        bcsb = consts.tile([128, 2 * NC1], F32)
        nc.sync.dma_start(bcsb[:], BC[:, :])
        nc.sync.dma_start(w1sb[:, :, 512:H1], W1r[:, :, 512:H1])
        w2sb = consts.tile([128, NC1, D], FP8)
        W2r = W2.rearrange("(kc p) d -> p kc d", p=128)
        nc.sync.dma_start(w2sb[:], W2r[:])
        h2ctsb = acts.tile([128, DS, RPC], BF16)
        H2CTr = H2CT.rearrange("(dc p) r -> p dc r", p=128)
        nc.sync.dma_start(h2ctsb[:], H2CTr[:])

        b1sb = bcsb[:, 0:NC1]
        tcsb = bcsb[:, NC1 : 2 * NC1]
        YTr = YT.rearrange("(dc p) r -> dc p r", p=128)

        # tv stored transposed: [j-in-chunk, j-chunk, row], fp8
        tvsb = acts.tile([128, NC1, RPC], FP8)

        # --- mm1: zT[j, r] = sum_k W1[k, j] h2T[k, r], fp8 DoubleRow ------
        for jc in range(NC1):
            for rg in range(RG):
                zp = zpsum.tile([128, 512], F32, tag=f"z{rg}")
                rs = rg * 512
                for kp in range(KC // 2):
                    nc.tensor.matmul(
                        zp[:],
                        w1sb[:, 2 * kp : 2 * kp + 2, jc * 128 : (jc + 1) * 128],
                        h2tsb[:, 2 * kp : 2 * kp + 2, rs : rs + 512],
                        start=(kp == 0),
                        stop=(kp == KC // 2 - 1),
                        perf_mode=DR,
                    )
                u = upool.tile([128, 512], F32, tag=f"u{rg}")
                nc.scalar.activation(
                    u[:],
                    zp[:],
                    mybir.ActivationFunctionType.Relu,
                    bias=b1sb[:, jc : jc + 1],
                    scale=1.0,
                )
                nc.vector.tensor_scalar(
                    tvsb[:, jc, rs : rs + 512],
                    u[:],
                    tcsb[:, jc : jc + 1],
                    None,
                    mybir.AluOpType.subtract,
                )

        # --- mm2: fT[d, r] = sum_j W2[j, d] tv[j, r], fp8 DoubleRow -------
        for dc in range(DS):
            for rg in range(RG):
                fp = fpsum.tile([128, 512], F32, tag=f"f{rg}")
                rs = rg * 512
                for kp in range(NC1 // 2):
                    nc.tensor.matmul(
                        fp[:],
                        w2sb[:, 2 * kp : 2 * kp + 2, dc * 128 : (dc + 1) * 128],
                        tvsb[:, 2 * kp : 2 * kp + 2, rs : rs + 512],
                        start=(kp == 0),
                        stop=(kp == NC1 // 2 - 1),
                        perf_mode=DR,
                    )
                ysb = ypool.tile([128, 512], BF16, tag=f"y{rg}")
                nc.vector.tensor_tensor(
                    ysb[:], fp[:], h2ctsb[:, dc, rs : rs + 512], mybir.AluOpType.add
                )
                nc.sync.dma_start(YTr[dc, :, rs : rs + 512], ysb[:])
    nc.compile()
    return nc


_CACHE = {}


def _get_bass():
    if "nc" not in _CACHE:
        _CACHE["nc"] = build_bass()
    return _CACHE["nc"]


def _host_fold(inputs):
    """Fold attention shortcut + BNs into W1, b1, W2, h2, h2ct (float64)."""
    f = lambda k: inputs[k].astype(np.float64)
    h = f("h")
    a1 = f("bn1_g") / np.sqrt(f("bn1_v") + EPS)
    c1 = f("bn1_b") - f("bn1_m") * a1
    a2 = f("bn2_g") / np.sqrt(f("bn2_v") + EPS)
    c2 = f("bn2_b") - f("bn2_m") * a2

    hs = h.sum(axis=0)
    s = hs @ f("vw") + N * f("vb")          # column sums of v
    base = s @ f("ow") + f("ob")            # constant attention-out row
    d1 = base * a1 + c1                     # constant row of bn1(x)
    sP = a1 * a2

    W1 = (1.0 / a2)[:, None] * f("f1w")
    b1 = (d1 @ f("f1w") + f("f1b")).astype(np.float32)
    W2 = f("f2w") * a2[None, :]
    C = (d1 + f("f2b")) * a2 + c2

    # device computes tv = relu(z + b1_f32) - tc_f32 in f32, so use the
    # exact same f32 constants when folding tc @ W2 into h2ct
    tc = np.maximum(b1, 0.0)
    Cfull = C + tc.astype(np.float64) @ W2

    h2 = h * sP[None, :]
    pack = lambda v: v.reshape(H1 // 128, 128).T
    return {
        "W1": W1.astype(NPFP8),
        "bc": np.ascontiguousarray(np.concatenate([pack(b1), pack(tc)], axis=1)),
        "W2": W2.astype(NPFP8),
        "h2t": np.ascontiguousarray(h2.astype(NPFP8).T),          # [D, N]
        "h2ct": np.ascontiguousarray((h2 + Cfull[None, :]).T.astype(NPBF16)),
    }


def make_in_maps(inputs):
    hf = _host_fold(inputs)
    in_maps = []
    for c in range(NCORES):
        r0 = c * RPC
        in_maps.append(
            {
                "h2t": np.ascontiguousarray(hf["h2t"][:, r0 : r0 + RPC]),
                "h2ct": np.ascontiguousarray(hf["h2ct"][:, r0 : r0 + RPC]),
                "w1": hf["W1"],
                "w2": hf["W2"],
                "bc": hf["bc"],
            }
        )
    return in_maps


def kernel(**inputs):
    nc = _get_bass()
    in_maps = make_in_maps(inputs)
    res = run_bass_kernel_spmd(nc, in_maps, core_ids=list(range(NCORES)))
    return np.concatenate(
        [r["yt"].T.astype(np.float32) for r in res.results], axis=0
    )
